# revision 19
# baseline (speedup 1.0000x reference)
"""Trainium2 Bass kernel for nn_CrossAttention (linear/efficient attention).

Math: out = x + bo + x_flat @ W_attn where
  W_attn = sum_h Wq_h @ cm_h @ Wo_h,
  cm_h  = softmax_n(k_h)^T @ v_h,  k = ctx_flat @ Wk, v = ctx_flat @ Wv.
(The q projection folds into W_attn.)

Sharding: 8 cores = 4 batches x 2 token-halves. Each core computes partial
[num|den] softmax statistics over its 8192 tokens; a pairwise AllReduce
merges them; each core then applies W_attn to its own token half.

Dataflow: all big matmuls run fp8 DoubleRow (256-deep contraction per
pass). ctx/x arrive fp8 (+x fp16 for the residual) from the host; the
output is stored fp16 and widened on the host. The residual x + bo is
precomputed into the output buffers by ACT/DVE during the AllReduce
window, phase 2 then accumulates W_attn^T x on top. cm-stat matmuls are
software-pipelined one token-pair behind the projections so the PE never
stalls on exp/copy latency.
"""

import sys

if "/opt/trn_rl_repo" not in sys.path:
    sys.path.insert(0, "/opt/trn_rl_repo")

import numpy as np
import ml_dtypes

B = 4
C = 256          # channels (DIM)
N_FULL = 16384   # tokens per batch (128*128)
T = 8192         # tokens per core
HEADS = 8
DH = 64
INNER = 512
NCORES = 8
CHUNK = 2048
NCH = T // CHUNK      # 4
SUBS = CHUNK // 128   # 16

_CACHE: dict = {}
LAST_RESULTS = None   # BassKernelResults of the most recent run (for profiling)
TRACE = False         # set True before calling kernel() to capture a trace


def _build_nc():
    import concourse.mybir as mybir
    import concourse.tile as tile
    from concourse import bacc

    f32, f16, f8 = mybir.dt.float32, mybir.dt.float16, mybir.dt.float8e4
    AF = mybir.ActivationFunctionType
    DR = mybir.MatmulPerfMode.DoubleRow
    ADD = mybir.AluOpType.add

    nc = bacc.Bacc("TRN2", target_bir_lowering=False, debug=False)

    xh = nc.dram_tensor("xh", [C, T], f16, kind="ExternalInput")
    x8d = nc.dram_tensor("x8", [C, T], f8, kind="ExternalInput")
    ch = nc.dram_tensor("ch", [C, T], f8, kind="ExternalInput")
    wkv = nc.dram_tensor("wkv", [C, 2 * INNER], f8, kind="ExternalInput")
    wqt = nc.dram_tensor("wqt", [INNER, C], f16, kind="ExternalInput")
    wo = nc.dram_tensor("wo", [INNER, C], f16, kind="ExternalInput")
    bo = nc.dram_tensor("bo", [C, 1], f32, kind="ExternalInput")
    out = nc.dram_tensor("out", [C, T], f16, kind="ExternalOutput")

    xh_r = xh.ap().rearrange("(kc p) n -> p kc n", p=128)
    x8_r = x8d.ap().rearrange("(kc p) n -> p kc n", p=128)
    ch_r = ch.ap().rearrange("(kc p) n -> p kc n", p=128)
    out_r = out.ap().rearrange("(oc p) n -> p oc n", p=128)

    with tile.TileContext(nc) as tc:
        with (
            tc.tile_pool(name="wpool", bufs=1) as wpool,
            tc.tile_pool(name="spool", bufs=3) as spool,
            tc.tile_pool(name="ppool", bufs=4) as ppool,
            tc.tile_pool(name="x16pool", bufs=1) as x16pool,
            tc.tile_pool(name="x8pool", bufs=1) as x8pool,
            tc.tile_pool(name="obuf", bufs=1) as obuf,
            tc.tile_pool(name="dpool", bufs=1, space="DRAM") as dpool,
        ):
            def load_ctx8(ci):
                ctx8 = spool.tile([128, 2, CHUNK], f8, tag="ctx8", name="ctx8")
                nc.sync.dma_start(
                    ctx8[:], ch_r[:, :, ci * CHUNK : (ci + 1) * CHUNK]
                )
                return ctx8

            ctx8_next = load_ctx8(0)

            # ---- weights (already converted on host) ----
            wkv8 = wpool.tile([128, 2, 2 * INNER], f8)
            nc.sync.dma_start(wkv8[:], wkv.ap().rearrange("(kc p) o -> p kc o", p=128))
            wqt16 = wpool.tile([128, 4, C], f16)
            nc.sync.dma_start(
                wqt16[:], wqt.ap().rearrange("(hc p) i -> p hc i", p=128)
            )
            wo16 = wpool.tile([64, HEADS, C], f16)
            nc.sync.dma_start(wo16[:], wo.ap().rearrange("(h p) o -> p h o", p=64))
            bo_sb = wpool.tile([128, 2], f32)
            nc.sync.dma_start(bo_sb[:], bo.ap().rearrange("(oc p) x -> p (oc x)", p=128))

            # tiny dummy AllReduce issued up front: pays the collective
            # channel-setup latency while phase 1 runs
            ccw_in = dpool.tile([128, 4], f32)
            ccw_out = dpool.tile([128, 4], f32)
            nc.sync.dma_start(ccw_in[:, 0:2], bo_sb[:])
            nc.sync.dma_start(ccw_in[:, 2:4], bo_sb[:])
            nc.gpsimd.collective_compute(
                "AllReduce",
                mybir.AluOpType.add,
                replica_groups=[[0, 1], [2, 3], [4, 5], [6, 7]],
                ins=[ccw_in.opt()],
                outs=[ccw_out.opt()],
            )

            # ---- phase 1: accumulate per-head [num | den] over local tokens ----
            # cm_ps[hp] rows 0:64   = head 2hp   : cols 0:64 num, col 64 den
            #           rows 64:128 = head 2hp+1 : cols 65:129 num, col 129 den
            cm_sb = wpool.tile([128, 4, 130], f32)
            x16_tiles = []
            x8_tiles = []
            NPAIR = NCH * SUBS // 2

            with (
                tc.tile_pool(name="ps_cm", bufs=1, space="PSUM") as ps_cm,
                tc.tile_pool(name="ps_kv", bufs=2, space="PSUM") as ps_kv,
            ):
                cm_ps = [
                    ps_cm.tile([128, 130], f32, tag=f"cm{i}", name=f"cm{i}")
                    for i in range(4)
                ]

                def emit_cm(pair, idx):
                    kexp8, vcat8 = pair
                    for hp in range(4):
                        nc.tensor.matmul(
                            cm_ps[hp][:],
                            lhsT=kexp8[:, :, hp * 128 : (hp + 1) * 128],
                            rhs=vcat8[:, :, 2 * hp : 2 * hp + 2, :],
                            start=(idx == 0),
                            stop=(idx == NPAIR - 1),
                            perf_mode=DR,
                        )

                pend = None
                pair_idx = 0
                for ci in range(NCH):
                    ctx8 = ctx8_next
                    if ci + 1 < NCH:
                        ctx8_next = load_ctx8(ci + 1)
                    # phase-2 x tiles: fp8 for the matmul, f16 for the
                    # residual; both straight from HBM, kept resident
                    x16 = x16pool.tile(
                        [128, 2, CHUNK], f16, tag=f"x16_{ci}", name=f"x16_{ci}"
                    )
                    nc.sync.dma_start(
                        x16[:], xh_r[:, :, ci * CHUNK : (ci + 1) * CHUNK]
                    )
                    x16_tiles.append(x16)
                    x8 = x8pool.tile(
                        [128, 2, CHUNK], f8, tag=f"x8_{ci}", name=f"x8_{ci}"
                    )
                    nc.sync.dma_start(
                        x8[:], x8_r[:, :, ci * CHUNK : (ci + 1) * CHUNK]
                    )
                    x8_tiles.append(x8)

                    for s in range(SUBS):
                        tok = slice(s * 128, (s + 1) * 128)
                        half = s % 2
                        if half == 0:
                            kexp8 = ppool.tile([128, 2, INNER], f8, tag="kexp")
                            vcat8 = ppool.tile([128, 2, 8, 65], f8, tag="vcat")
                            nc.gpsimd.memset(vcat8[:, :, :, 64], 1.0)
                            # flush the PREVIOUS pair's cm matmuls now: its
                            # exp/copy ops had a full pair of time to finish,
                            # so the PE never stalls on them
                            if pend is not None:
                                emit_cm(pend, pair_idx - 1)
                        # K / V projections: contraction 256 in one
                        # DoubleRow pass each
                        k_ps = ps_kv.tile([128, INNER], f32, tag="k")
                        nc.tensor.matmul(
                            k_ps[:],
                            lhsT=ctx8[:, :, tok],
                            rhs=wkv8[:, :, 0:INNER],
                            start=True,
                            stop=True,
                            perf_mode=DR,
                        )
                        nc.scalar.activation(
                            kexp8[:, half, :], k_ps[:], AF.Exp
                        )
                        v_ps = ps_kv.tile([128, INNER], f32, tag="v")
                        nc.tensor.matmul(
                            v_ps[:],
                            lhsT=ctx8[:, :, tok],
                            rhs=wkv8[:, :, INNER : 2 * INNER],
                            start=True,
                            stop=True,
                            perf_mode=DR,
                        )
                        nc.vector.tensor_copy(
                            vcat8[:, half, :, 0:64],
                            v_ps[:].rearrange("p (h e) -> p h e", h=8),
                        )
                        if half == 1:
                            pend = (kexp8, vcat8)
                            pair_idx += 1
                emit_cm(pend, NPAIR - 1)
                for hp in range(4):
                    nc.vector.tensor_copy(cm_sb[:, hp, :], cm_ps[hp][:])

            # ---- pairwise AllReduce of [num|den] across the 2 token halves ----
            cc_in = dpool.tile([128, 4, 65], f32)
            cc_out = dpool.tile([128, 4, 65], f32)
            nc.sync.dma_start(cc_in[0:64, :, :], cm_sb[0:64, :, 0:65])
            nc.sync.dma_start(cc_in[64:128, :, :], cm_sb[64:128, :, 65:130])
            nc.gpsimd.collective_compute(
                "AllReduce",
                mybir.AluOpType.add,
                replica_groups=[[0, 1], [2, 3], [4, 5], [6, 7]],
                ins=[cc_in.opt()],
                outs=[cc_out.opt()],
            )
            mm_sb = wpool.tile([128, 4, 65], f32)
            nc.sync.dma_start(mm_sb[:], cc_out[:])

            # ---- residual precompute: obuf[ci] = x + bo, during the CC
            # window (ACT and DVE are otherwise idle there). bo_gate is
            # bo + 0*cm_sb: the fake cm_sb read pins these ops AFTER
            # phase 1 so the scheduler cannot hoist them into the middle
            # of the exp stream ----
            bo_gate = wpool.tile([128, 2], f32)
            nc.vector.scalar_tensor_tensor(
                bo_gate[:],
                in0=cm_sb[:, 0, 0:2],
                scalar=0.0,
                in1=bo_sb[:],
                op0=mybir.AluOpType.mult,
                op1=mybir.AluOpType.add,
            )
            out_tiles = []
            for ci in range(NCH):
                ot = obuf.tile([128, 2, CHUNK], f16, tag=f"out_{ci}", name=f"out_{ci}")
                out_tiles.append(ot)
                for oc in range(2):
                    if (ci + oc) % 2 == 0:
                        nc.scalar.activation(
                            ot[:, oc, :],
                            x16_tiles[ci][:, oc, :],
                            AF.Identity,
                            bias=bo_gate[:, oc : oc + 1],
                        )
                    else:
                        nc.vector.tensor_scalar_add(
                            ot[:, oc, :],
                            x16_tiles[ci][:, oc, :],
                            bo_gate[:, oc : oc + 1],
                        )

            # keep the PE (and die) clocked through the AllReduce window
            with tc.tile_pool(name="ps_warm", bufs=1, space="PSUM") as ps_warm:
                warm_ps = ps_warm.tile(
                    [128, 2 * C], f32, tag="warm", name="warm_ps"
                )
                for _ in range(94):
                    nc.tensor.matmul(
                        warm_ps[:],
                        lhsT=wqt16[:, 0, 0:128],
                        rhs=wqt16[:, 0:2, :],
                        start=True,
                        stop=True,
                    )

            # ---- normalize cm, build W_attn = sum_h Wq_h cm_h Wo_h (fp8) ----
            deninv = wpool.tile([128, 4], f32)
            cmn16 = wpool.tile([128, 4, 64], f16)
            m1t16 = wpool.tile([64, 8, C], f16)
            wattn8 = wpool.tile([128, 2, C], f8)
            with (
                tc.tile_pool(name="ps_post", bufs=2, space="PSUM") as ps_post,
                tc.tile_pool(name="ps_o", bufs=4, space="PSUM") as ps_o,
            ):
                nc.vector.reciprocal(deninv[:], mm_sb[:, :, 64])
                for hp in range(4):
                    nc.vector.tensor_scalar_mul(
                        cmn16[:, hp, :],
                        mm_sb[:, hp, 0:64],
                        deninv[:, hp : hp + 1],
                    )
                for h in range(HEADS):
                    hp, hh = h // 2, h % 2
                    rs = slice(hh * 64, hh * 64 + 64)
                    m1t_ps = ps_post.tile([64, C], f32, tag="m1t")
                    nc.tensor.matmul(
                        m1t_ps[:],
                        lhsT=cmn16[rs, hp, :],
                        rhs=wqt16[rs, hp, :],
                        start=True,
                        stop=True,
                    )
                    nc.vector.tensor_copy(m1t16[:, h, :], m1t_ps[:])
                for ic in range(2):
                    weff_ps = ps_post.tile([128, C], f32, tag="weff")
                    for h in range(HEADS):
                        nc.tensor.matmul(
                            weff_ps[:],
                            lhsT=m1t16[:, h, ic * 128 : (ic + 1) * 128],
                            rhs=wo16[:, h, :],
                            start=(h == 0),
                            stop=(h == HEADS - 1),
                        )
                    nc.vector.tensor_copy(wattn8[:, ic, :], weff_ps[:])

                # ---- phase 2: obuf += W_attn^T x (fp8 DoubleRow), store ----
                NH = CHUNK // 512
                for ci in range(NCH):
                    x8 = x8_tiles[ci]
                    ot = out_tiles[ci]
                    for oc in range(2):
                        for nh in range(NH):
                            ts_ = slice(nh * 512, (nh + 1) * 512)
                            o_ps = ps_o.tile([128, 512], f32, tag="o")
                            nc.tensor.matmul(
                                o_ps[:],
                                lhsT=wattn8[:, :, oc * 128 : (oc + 1) * 128],
                                rhs=x8[:, :, ts_],
                                start=True,
                                stop=True,
                                perf_mode=DR,
                            )
                            # accumulate onto the precomputed residual;
                            # alternate DVE / ACT+GPSIMD to balance engines
                            if nh % 2 == 0:
                                nc.vector.tensor_tensor(
                                    ot[:, oc, ts_], o_ps[:], ot[:, oc, ts_], ADD
                                )
                            else:
                                tmp16 = spool.tile([128, 512], f16, tag="tmp")
                                nc.scalar.copy(tmp16[:], o_ps[:])
                                nc.gpsimd.tensor_add(
                                    ot[:, oc, ts_], tmp16[:], ot[:, oc, ts_]
                                )
                        nc.sync.dma_start(
                            out_r[:, oc, ci * CHUNK : (ci + 1) * CHUNK],
                            ot[:, oc, :],
                        )

    nc.compile()
    return nc


def _get_nc():
    if "nc" not in _CACHE:
        _CACHE["nc"] = _build_nc()
    return _CACHE["nc"]


def kernel(**inputs) -> np.ndarray:
    global LAST_RESULTS
    from concourse.bass_utils import run_bass_kernel_spmd

    f8 = ml_dtypes.float8_e4m3
    x = np.ascontiguousarray(np.asarray(inputs["x"], dtype=np.float32))
    ctx = np.ascontiguousarray(np.asarray(inputs["context"], dtype=np.float32))
    Wq = np.asarray(inputs["Wq"], dtype=np.float32)
    Wk = np.asarray(inputs["Wk"], dtype=np.float32)
    Wv = np.asarray(inputs["Wv"], dtype=np.float32)
    Wo = np.asarray(inputs["Wo"], dtype=np.float32)
    bo = np.ascontiguousarray(
        np.asarray(inputs["bo"], dtype=np.float32).reshape(C, 1)
    )
    wkv8 = np.ascontiguousarray(
        np.concatenate([Wk, Wv], axis=1).astype(f8)
    )
    wqt16 = np.ascontiguousarray(Wq.T.astype(np.float16))
    wo16 = np.ascontiguousarray(Wo.astype(np.float16))

    x16 = x.reshape(B, C, N_FULL).astype(np.float16)
    x8f = x.reshape(B, C, N_FULL).astype(f8)
    c8 = ctx.reshape(B, C, N_FULL).astype(f8)

    in_maps = []
    for c in range(NCORES):
        b, t = c // 2, c % 2
        sl = slice(t * T, (t + 1) * T)
        in_maps.append(
            {
                "xh": np.ascontiguousarray(x16[b, :, sl]),
                "x8": np.ascontiguousarray(x8f[b, :, sl]),
                "ch": np.ascontiguousarray(c8[b, :, sl]),
                "wkv": wkv8,
                "wqt": wqt16,
                "wo": wo16,
                "bo": bo,
            }
        )

    nc = _get_nc()
    res = run_bass_kernel_spmd(nc, in_maps, list(range(NCORES)), trace=TRACE)
    LAST_RESULTS = res

    out = np.empty((B, C, N_FULL), dtype=np.float32)
    for c in range(NCORES):
        b, t = c // 2, c % 2
        out[b, :, t * T : (t + 1) * T] = res.results[c]["out"].astype(np.float32)
    return out.reshape(B, C, 128, 128)
